# revision 6
# baseline (speedup 1.0000x reference)
"""Trainium2 Bass kernel for Conv1D-MHSA with p4-sketch linear attention.

Math: the reference computes
    scores = phi_q @ phi_k^T            # [B,H,L,L]
    attn   = scores / (scores.sum(-1) + 1e-6)
    o      = attn @ v
Since phi_* >= 0 and the normalizer is a plain row sum, this reassociates
exactly to linear attention:
    M   = phi_k^T @ v                   # [R, HD] per (b, h)
    s   = phi_k.sum(axis=L)             # [R]
    o   = (phi_q @ M) / (phi_q @ s + 1e-6)
which removes the [L, L] score materialization entirely.

Sharding: 8 cores = (batch b in {0,1}) x (L-quarter j in {0..3}).
Each core runs the causal convs (2-col halo), sketches, and the output
projection for its 512 positions.  The only cross-core data is the
[R x (HD+1)] per-head M/s reduction over L -> one 68 KB AllReduce per
batch group ([[0,1,2,3],[4,5,6,7]]).

Layout trick: heads are processed 4-at-a-time (4 x HD=32 = 128 partitions)
using block-diagonal sketch matrices (gamma and R^-1/4 folded in on host),
so every matmul is a full 128-contract matmul.
"""

import math
import os

import numpy as np

import concourse.bass as bass
import concourse.tile as tile
from concourse import bacc, mybir
from concourse.bass_utils import run_bass_kernel_spmd

F32 = mybir.dt.float32

B, DIM, L = 2, 512, 2048
H, HD, R, K = 16, 32, 32, 3
NCORES = 8
CHUNK = L // 4          # 512 positions per core
NG = DIM // 128         # 4 channel groups (4 heads each)
NT = CHUNK // 128       # 4 position tiles per core
AF = mybir.ActivationFunctionType


def _build_nc():
    nc = bacc.Bacc("TRN2", debug=False, num_devices=NCORES)

    # Per-core DRAM I/O (names must match in_maps keys)
    x_sl = nc.dram_tensor("x_sl", [DIM, CHUNK + K - 1], F32, kind="ExternalInput").ap()
    qw = nc.dram_tensor("qw", [K, DIM, DIM], F32, kind="ExternalInput").ap()
    kw = nc.dram_tensor("kw", [K, DIM, DIM], F32, kind="ExternalInput").ap()
    vw = nc.dram_tensor("vw", [DIM, DIM], F32, kind="ExternalInput").ap()
    pw = nc.dram_tensor("pw", [DIM, DIM], F32, kind="ExternalInput").ap()
    bdg = nc.dram_tensor("bdg", [4, 128, 128], F32, kind="ExternalInput").ap()
    qb = nc.dram_tensor("qb", [NG, 128], F32, kind="ExternalInput").ap()
    kb = nc.dram_tensor("kb", [NG, 128], F32, kind="ExternalInput").ap()
    pb = nc.dram_tensor("pb", [DIM], F32, kind="ExternalInput").ap()
    out = nc.dram_tensor("out", [CHUNK, DIM], F32, kind="ExternalOutput").ap()

    with tile.TileContext(nc) as tc:
        with (
            tc.tile_pool(name="consts", bufs=1) as consts,
            tc.tile_pool(name="work", bufs=1) as work,
            tc.tile_pool(name="s128", bufs=2) as s128,
            tc.tile_pool(name="s512", bufs=2) as s512,
            tc.tile_pool(name="p512", bufs=3, space="PSUM") as p512,
            tc.tile_pool(name="p128", bufs=4, space="PSUM") as p128,
            tc.tile_pool(name="ppg", bufs=1, space="PSUM") as ppg,
            tc.tile_pool(name="dram", bufs=1, space="DRAM") as dpool,
        ):
            # ---- constant loads (k-path first so PE can start ASAP) ----
            xs = consts.tile([128, NG, CHUNK + K - 1], F32)
            nc.sync.dma_start(out=xs[:], in_=x_sl.rearrange("(ct p) t -> p ct t", p=128))
            kws = consts.tile([128, K, NG, DIM], F32)
            nc.sync.dma_start(out=kws[:], in_=kw.rearrange("k (ct p) n -> p k ct n", p=128))
            vws = consts.tile([128, NG, DIM], F32)
            nc.sync.dma_start(out=vws[:], in_=vw.rearrange("(ct p) n -> p ct n", p=128))
            bdgs = consts.tile([128, 4, 128], F32)
            nc.sync.dma_start(out=bdgs[:], in_=bdg.rearrange("g p n -> p g n"))
            kbs = consts.tile([128, NG], F32)
            nc.sync.dma_start(out=kbs[:], in_=kb.rearrange("ct p -> p ct"))
            qws = consts.tile([128, K, NG, DIM], F32)
            nc.sync.dma_start(out=qws[:], in_=qw.rearrange("k (ct p) n -> p k ct n", p=128))
            qbs = consts.tile([128, NG], F32)
            nc.sync.dma_start(out=qbs[:], in_=qb.rearrange("ct p -> p ct"))
            pws = consts.tile([128, NG, DIM], F32)
            nc.sync.dma_start(out=pws[:], in_=pw.rearrange("(g p) n -> p g n", p=128))
            pbs = consts.tile([128, DIM], F32)
            pb_bcast = bass.AP(tensor=pb.tensor, offset=pb.offset, ap=[[0, 128], *pb.ap])
            nc.sync.dma_start(out=pbs[:], in_=pb_bcast)

            ones = consts.tile([128, 32], F32)
            nc.vector.memset(ones[:], 1.0)

            # ---- k conv: k[c, t] = sum_tap sum_cin Wk[tap][c, cin] x[cin, t+tap-2]
            k_sb = work.tile([128, NG, CHUNK], F32)
            for g in range(NG):
                kps = p512.tile([128, CHUNK], F32, tag="mm")
                idx = 0
                for tap in range(K):
                    for ct in range(NG):
                        nc.tensor.matmul(
                            kps[:],
                            lhsT=kws[:, tap, ct, g * 128:(g + 1) * 128],
                            rhs=xs[:, ct, tap:tap + CHUNK],
                            start=(idx == 0),
                            stop=(idx == K * NG - 1),
                        )
                        idx += 1
                nc.scalar.activation(k_sb[:, g, :], kps[:], AF.Identity,
                                     bias=kbs[:, g:g + 1], scale=1.0)

            # ---- v^T: vT[t, c] = sum_cin x[cin, t] Wv^T[cin, c]; ones col at 128
            vT_sb = work.tile([128, NT, NG, 129], F32)
            nc.vector.memset(vT_sb[:], 1.0)
            for tau in range(NT):
                vps = p512.tile([128, DIM], F32, tag="mm")
                for ct in range(NG):
                    nc.tensor.matmul(
                        vps[:],
                        lhsT=xs[:, ct, (K - 1) + tau * 128:(K - 1) + tau * 128 + 128],
                        rhs=vws[:, ct, :],
                        start=(ct == 0),
                        stop=(ct == NG - 1),
                    )
                for g in range(NG):
                    nc.vector.tensor_copy(vT_sb[:, tau, g, 0:128],
                                          vps[:, g * 128:(g + 1) * 128])

            # ---- phi_k in [t, (h,r)] layout: (k^T @ BDG1k * k^T @ BDG2k)^2
            phik = work.tile([128, NT, NG, 128], F32)
            for g in range(NG):
                for tau in range(NT):
                    a1 = p128.tile([128, 128], F32, tag="ak")
                    a2 = p128.tile([128, 128], F32, tag="ak")
                    ksl = k_sb[:, g, tau * 128:(tau + 1) * 128]
                    nc.tensor.matmul(a1[:], lhsT=ksl, rhs=bdgs[:, 2, :], start=True, stop=True)
                    nc.tensor.matmul(a2[:], lhsT=ksl, rhs=bdgs[:, 3, :], start=True, stop=True)
                    s1 = s128.tile([128, 128], F32, tag="ks")
                    t1 = s128.tile([128, 128], F32, tag="pkt")
                    nc.scalar.copy(s1[:], a1[:])  # DVE can't read 2 PSUM operands
                    nc.vector.tensor_mul(t1[:], s1[:], a2[:])
                    nc.vector.tensor_mul(phik[:, tau, g, :], t1[:], t1[:])

            # ---- M-cross + s: pg[(h,r), (h',d)|s] = sum_t phik[t,(h,r)] vT[t,(h',d)|1]
            mcomp = work.tile([128, NG, 33], F32)
            for g in range(NG):
                pg = ppg.tile([128, 129], F32, tag="pg")
                for tau in range(NT):
                    nc.tensor.matmul(pg[:], lhsT=phik[:, tau, g, :],
                                     rhs=vT_sb[:, tau, g, :],
                                     start=(tau == 0), stop=(tau == NT - 1))
                for hg in range(4):
                    sl = slice(32 * hg, 32 * hg + 32)
                    nc.vector.tensor_copy(mcomp[sl, g, 0:32], pg[sl, sl])
                nc.vector.tensor_copy(mcomp[:, g, 32:33], pg[:, 128:129])

            # ---- AllReduce M/s across the 4 L-shards of this batch ----
            ccin = dpool.tile([128, NG, 33], F32)
            ccout = dpool.tile([128, NG, 33], F32)
            nc.sync.dma_start(out=ccin[:], in_=mcomp[:])
            nc.gpsimd.collective_compute(
                "AllReduce",
                mybir.AluOpType.add,
                replica_groups=[[0, 1, 2, 3], [4, 5, 6, 7]],
                ins=[ccin.opt()],
                outs=[ccout.opt()],
            )
            mred = work.tile([128, NG, 33], F32)
            nc.sync.dma_start(out=mred[:], in_=ccout[:])

            # ---- q conv (overlaps the collective) ----
            q_sb = work.tile([128, NG, CHUNK], F32)
            for g in range(NG):
                qps = p512.tile([128, CHUNK], F32, tag="mm")
                idx = 0
                for tap in range(K):
                    for ct in range(NG):
                        nc.tensor.matmul(
                            qps[:],
                            lhsT=qws[:, tap, ct, g * 128:(g + 1) * 128],
                            rhs=xs[:, ct, tap:tap + CHUNK],
                            start=(idx == 0),
                            stop=(idx == K * NG - 1),
                        )
                        idx += 1
                nc.scalar.activation(q_sb[:, g, :], qps[:], AF.Identity,
                                     bias=qbs[:, g:g + 1], scale=1.0)

            # ---- phi_q in [(h,r), t] layout ----
            phiq = work.tile([128, NG, CHUNK], F32)
            for g in range(NG):
                a1q = p512.tile([128, CHUNK], F32, tag="mm")
                a2q = p512.tile([128, CHUNK], F32, tag="mm")
                nc.tensor.matmul(a1q[:], lhsT=bdgs[:, 0, :], rhs=q_sb[:, g, :], start=True, stop=True)
                nc.tensor.matmul(a2q[:], lhsT=bdgs[:, 1, :], rhs=q_sb[:, g, :], start=True, stop=True)
                sq = s512.tile([128, CHUNK], F32, tag="sq")
                tq = s512.tile([128, CHUNK], F32, tag="tq")
                nc.scalar.copy(sq[:], a1q[:])  # DVE can't read 2 PSUM operands
                nc.vector.tensor_mul(tq[:], sq[:], a2q[:])
                nc.vector.tensor_mul(phiq[:, g, :], tq[:], tq[:])

            # ---- block-diag M (bd) and row-replicated s (srep) from reduced stats
            bd = work.tile([128, NG, 128], F32)
            srep = work.tile([128, NG, 128], F32)
            nc.vector.memset(bd[:], 0.0)
            nc.vector.memset(srep[:], 0.0)
            for g in range(NG):
                for hg in range(4):
                    sl = slice(32 * hg, 32 * hg + 32)
                    nc.vector.tensor_copy(bd[sl, g, sl], mred[sl, g, 0:32])
                    nc.vector.tensor_scalar_mul(srep[sl, g, sl], ones[sl, 0:32],
                                                mred[sl, g, 32:33])

            # ---- numerator / denominator / o ----
            o_sb = work.tile([128, NG, CHUNK], F32)
            for g in range(NG):
                nps = p512.tile([128, CHUNK], F32, tag="mm")
                dps = p512.tile([128, CHUNK], F32, tag="mm")
                nc.tensor.matmul(nps[:], lhsT=bd[:, g, :], rhs=phiq[:, g, :], start=True, stop=True)
                nc.tensor.matmul(dps[:], lhsT=srep[:, g, :], rhs=phiq[:, g, :], start=True, stop=True)
                dsb = s512.tile([128, CHUNK], F32, tag="dsb")
                rsb = s512.tile([128, CHUNK], F32, tag="rsb")
                nc.vector.tensor_scalar_add(dsb[:], dps[:], 1e-6)
                nc.vector.reciprocal(rsb[:], dsb[:])
                nc.vector.tensor_mul(o_sb[:, g, :], nps[:], rsb[:])

            # ---- output projection: out[t, i] = sum_c o[c, t] projT[c, i] + pb[i]
            outs_sb = work.tile([128, NT, DIM], F32)
            for tau in range(NT):
                ops = p512.tile([128, DIM], F32, tag="mm")
                for g in range(NG):
                    nc.tensor.matmul(
                        ops[:],
                        lhsT=o_sb[:, g, tau * 128:(tau + 1) * 128],
                        rhs=pws[:, g, :],
                        start=(g == 0),
                        stop=(g == NG - 1),
                    )
                nc.vector.tensor_add(outs_sb[:, tau, :], ops[:], pbs[:])
            nc.sync.dma_start(out=out.rearrange("(tau p) i -> p tau i", p=128),
                              in_=outs_sb[:])

    nc.compile()
    return nc


_NC_CACHE = {}


def _get_nc():
    if "nc" not in _NC_CACHE:
        _NC_CACHE["nc"] = _build_nc()
    return _NC_CACHE["nc"]


def _numpy_fallback(x, q_w, q_b, k_w, k_b, v_w, proj_w, proj_b,
                    gamma_q, beta_q, gamma_k, beta_k, G1q, G2q, G1k, G2k):
    # Exact-reference path, only taken for parameter regimes the Bass kernel
    # doesn't specialize for (beta != 0). Never hit with the shipped setup.
    xp = np.pad(x, ((0, 0), (0, 0), (K - 1, 0)))
    def conv(xx, w):
        o = np.zeros((B, w.shape[0], L), np.float32)
        for t in range(w.shape[2]):
            o += np.einsum("oi,bit->bot", w[:, :, t], xx[:, :, t:t + L])
        return o
    q = conv(xp, q_w) + q_b[None, :, None]
    k = conv(xp, k_w) + k_b[None, :, None]
    v = np.einsum("oi,bit->bot", v_w[:, :, 0], x)
    def shp(t):
        return t.reshape(B, H, HD, L).transpose(0, 1, 3, 2)
    q, k, v = shp(q), shp(k), shp(v)
    q = gamma_q * q + beta_q
    k = gamma_k * k + beta_k
    def sk(t, G1, G2):
        half = (t @ G1) * (t @ G2) / math.sqrt(R)
        return half * half
    pq, pk = sk(q, G1q, G2q), sk(k, G1k, G2k)
    M = np.einsum("bhlr,bhld->bhrd", pk, v)
    s = pk.sum(axis=2)
    num = np.einsum("bhlr,bhrd->bhld", pq, M)
    den = np.einsum("bhlr,bhr->bhl", pq, s) + 1e-6
    o = num / den[..., None]
    o = o.transpose(0, 1, 3, 2).reshape(B, DIM, L).transpose(0, 2, 1)
    return (o @ proj_w.T + proj_b).astype(np.float32)


def kernel(**inputs):
    f = lambda k_: np.ascontiguousarray(np.asarray(inputs[k_], dtype=np.float32))
    x, q_w, q_b, k_w, k_b = f("x"), f("q_w"), f("q_b"), f("k_w"), f("k_b")
    v_w, proj_w, proj_b = f("v_w"), f("proj_w"), f("proj_b")
    G1q, G2q, G1k, G2k = f("G1q"), f("G2q"), f("G1k"), f("G2k")
    gamma_q = float(np.asarray(inputs["gamma_q"]).reshape(-1)[0])
    beta_q = float(np.asarray(inputs["beta_q"]).reshape(-1)[0])
    gamma_k = float(np.asarray(inputs["gamma_k"]).reshape(-1)[0])
    beta_k = float(np.asarray(inputs["beta_k"]).reshape(-1)[0])

    if beta_q != 0.0 or beta_k != 0.0:
        return _numpy_fallback(x, q_w, q_b, k_w, k_b, v_w, proj_w, proj_b,
                               gamma_q, beta_q, gamma_k, beta_k,
                               G1q, G2q, G1k, G2k)

    # host-side weight prep
    cfac = R ** (-0.25)
    g_mats = [G1q * (gamma_q * cfac), G2q * (gamma_q * cfac),
              G1k * (gamma_k * cfac), G2k * (gamma_k * cfac)]
    bdg = np.zeros((4, 128, 128), np.float32)
    for gi, gm in enumerate(g_mats):
        for i in range(4):
            bdg[gi, 32 * i:32 * i + 32, 32 * i:32 * i + 32] = gm
    common = dict(
        qw=np.ascontiguousarray(q_w.transpose(2, 1, 0)),    # [K, cin, cout]
        kw=np.ascontiguousarray(k_w.transpose(2, 1, 0)),
        vw=np.ascontiguousarray(v_w[:, :, 0].T),            # [cin, cout]
        pw=np.ascontiguousarray(proj_w.T),                  # [c, i]
        bdg=bdg,
        qb=np.ascontiguousarray(q_b.reshape(NG, 128)),
        kb=np.ascontiguousarray(k_b.reshape(NG, 128)),
        pb=proj_b,
    )
    xpad = np.pad(x, ((0, 0), (0, 0), (K - 1, 0)))          # [B, DIM, L+2]
    in_maps = []
    for core in range(NCORES):
        b, j = divmod(core, 4)
        xsl = np.ascontiguousarray(xpad[b][:, j * CHUNK: j * CHUNK + CHUNK + K - 1])
        in_maps.append(dict(x_sl=xsl, **common))

    nc = _get_nc()
    res = run_bass_kernel_spmd(nc, in_maps, list(range(NCORES)),
                               trace=bool(os.environ.get("BASS_TRACE")))
    kernel.last_results = res

    out = np.empty((B, L, DIM), np.float32)
    for core in range(NCORES):
        b, j = divmod(core, 4)
        out[b, j * CHUNK:(j + 1) * CHUNK, :] = res.results[core]["out"]
    return out


# revision 17
# speedup vs baseline: 1.8558x; 1.8558x over previous
"""Trainium2 Bass kernel for Conv1D-MHSA with p4-sketch linear attention.

Math: the reference computes
    scores = phi_q @ phi_k^T            # [B,H,L,L]
    attn   = scores / (scores.sum(-1) + 1e-6)
    o      = attn @ v
Since phi_* >= 0 and the normalizer is a plain row sum, this reassociates
exactly to linear attention:
    M   = phi_k^T @ v                   # [R, HD] per (b, h)
    s   = phi_k.sum(axis=L)             # [R]
    o   = (phi_q @ M) / (phi_q @ s + 1e-6)
which removes the [L, L] score materialization entirely.

Sharding: 8 cores = (batch b in {0,1}) x (L-quarter j in {0..3}).
Each core runs the causal convs (2-col halo), sketches, and the output
projection for its 512 positions.  The only cross-core data is the
[R x (HD+1)] per-head M/s reduction over L -> one 68 KB AllGather per
batch group ([[0,1,2,3],[4,5,6,7]]) + on-chip shard sum.

Layout trick: heads are processed 4-at-a-time (4 x HD=32 = 128 partitions)
using block-diagonal sketch matrices (gamma and R^-1/4 folded in on host),
so every matmul is a full 128-contract matmul.  Matmuls run in float32r
(single-pass fp32) instead of float32 (two-pass).
"""

import math
import os

import numpy as np

import concourse.bass as bass
import concourse.tile as tile
from concourse import bacc, mybir
from concourse.bass_utils import run_bass_kernel_spmd

F32 = mybir.dt.float32
F32R = mybir.dt.float32r

B, DIM, L = 2, 512, 2048
H, HD, R, K = 16, 32, 32, 3
NCORES = 8
CHUNK = L // 4          # 512 positions per core
NG = DIM // 128         # 4 channel groups (4 heads each)
NT = CHUNK // 128       # 4 position tiles per core
AF = mybir.ActivationFunctionType


def _build_nc():
    nc = bacc.Bacc("TRN2", debug=False, num_devices=NCORES)

    # Per-core DRAM I/O (names must match in_maps keys)
    x_sl = nc.dram_tensor("x_sl", [DIM, CHUNK + K - 1], F32R, kind="ExternalInput").ap()
    qw = nc.dram_tensor("qw", [K, DIM, DIM], F32R, kind="ExternalInput").ap()
    kw = nc.dram_tensor("kw", [K, DIM, DIM], F32R, kind="ExternalInput").ap()
    vw = nc.dram_tensor("vw", [DIM, DIM], F32R, kind="ExternalInput").ap()
    pw = nc.dram_tensor("pw", [DIM, DIM], F32R, kind="ExternalInput").ap()
    bdg = nc.dram_tensor("bdg", [4, 128, 128], F32R, kind="ExternalInput").ap()
    qb = nc.dram_tensor("qb", [NG, 128], F32, kind="ExternalInput").ap()
    kb = nc.dram_tensor("kb", [NG, 128], F32, kind="ExternalInput").ap()
    pb = nc.dram_tensor("pb", [DIM], F32, kind="ExternalInput").ap()
    out = nc.dram_tensor("out", [CHUNK, DIM], F32, kind="ExternalOutput").ap()

    with tile.TileContext(nc) as tc:
        with (
            tc.tile_pool(name="consts", bufs=1) as consts,
            tc.tile_pool(name="work", bufs=1) as work,
            tc.tile_pool(name="s128", bufs=2) as s128,
            tc.tile_pool(name="s512", bufs=2) as s512,
            tc.tile_pool(name="p512", bufs=3, space="PSUM") as p512,
            tc.tile_pool(name="p128", bufs=4, space="PSUM") as p128,
            tc.tile_pool(name="ppg", bufs=1, space="PSUM") as ppg,
            tc.tile_pool(name="dram", bufs=1, space="DRAM") as dpool,
        ):
            # ---- loads, in need-order (sync ring drains FIFO) ----
            xs = consts.tile([128, NG, CHUNK + K - 1], F32R)
            nc.sync.dma_start(out=xs[:], in_=x_sl.rearrange("(ct p) t -> p ct t", p=128))
            vws = consts.tile([128, NG, DIM], F32R)
            nc.sync.dma_start(out=vws[:], in_=vw.rearrange("(ct p) n -> p ct n", p=128))
            kws = consts.tile([128, K, NG, DIM], F32R)
            nc.sync.dma_start(out=kws[:], in_=kw.rearrange("k (ct p) n -> p k ct n", p=128))
            bdgs = consts.tile([128, 4, 128], F32R)
            nc.sync.dma_start(out=bdgs[:], in_=bdg.rearrange("g p n -> p g n"))
            kbs = consts.tile([128, NG], F32)
            nc.sync.dma_start(out=kbs[:], in_=kb.rearrange("ct p -> p ct"))
            qws = consts.tile([128, K, NG, DIM], F32R)
            nc.sync.dma_start(out=qws[:], in_=qw.rearrange("k (ct p) n -> p k ct n", p=128))
            qbs = consts.tile([128, NG], F32)
            nc.sync.dma_start(out=qbs[:], in_=qb.rearrange("ct p -> p ct"))
            pws = consts.tile([128, NG, DIM], F32R)
            nc.sync.dma_start(out=pws[:], in_=pw.rearrange("(g p) n -> p g n", p=128))
            pbs = consts.tile([128, DIM], F32)
            pb_bcast = bass.AP(tensor=pb.tensor, offset=pb.offset, ap=[[0, 128], *pb.ap])
            nc.sync.dma_start(out=pbs[:], in_=pb_bcast)

            # f32 scratch for filling f32r tiles (memset can't write f32r)
            onesf = consts.tile([128, 32], F32)
            nc.vector.memset(onesf[:], 1.0)
            zerof = consts.tile([128, 512], F32)
            nc.vector.memset(zerof[:], 0.0)

            # ---- v^T first: vT[t, c] = sum_cin x[cin, t] Wv^T[cin, c]; ones cols at 128:130
            # (two ones columns: fp32r matmul free dims must be even)
            vT_sb = work.tile([128, NT, NG, 130], F32R)
            nc.scalar.copy(vT_sb[:, :, :, 128:130],
                           onesf[:, 0:32].rearrange("p (a b c) -> p a b c", a=NT, b=NG))
            for tau in range(NT):
                vps = p512.tile([128, DIM], F32, tag="mm")
                for ct in range(NG):
                    nc.tensor.matmul(
                        vps[:],
                        lhsT=xs[:, ct, (K - 1) + tau * 128:(K - 1) + tau * 128 + 128],
                        rhs=vws[:, ct, :],
                        start=(ct == 0),
                        stop=(ct == NG - 1),
                    )
                for g in range(NG):
                    nc.vector.tensor_copy(vT_sb[:, tau, g, 0:128],
                                          vps[:, g * 128:(g + 1) * 128])

            # ---- per channel-group: k conv -> phi_k -> M/s partial ----
            k_sb = work.tile([128, NG, CHUNK], F32R)
            phik = work.tile([128, NT, NG, 128], F32R)
            mcomp = work.tile([128, NG, 33], F32)
            for g in range(NG):
                kps = p512.tile([128, CHUNK], F32, tag="mm")
                idx = 0
                for tap in range(K):
                    for ct in range(NG):
                        nc.tensor.matmul(
                            kps[:],
                            lhsT=kws[:, tap, ct, g * 128:(g + 1) * 128],
                            rhs=xs[:, ct, tap:tap + CHUNK],
                            start=(idx == 0),
                            stop=(idx == K * NG - 1),
                        )
                        idx += 1
                nc.scalar.activation(k_sb[:, g, :], kps[:], AF.Identity,
                                     bias=kbs[:, g:g + 1], scale=1.0)

                # phi_k in [t, (h,r)] layout: ((k^T BDG1k) * (k^T BDG2k))^2
                for tau in range(NT):
                    a12 = p128.tile([128, 256], F32, tag="ak")
                    ksl = k_sb[:, g, tau * 128:(tau + 1) * 128]
                    nc.tensor.matmul(a12[:], lhsT=ksl, rhs=bdgs[:, 2:4, :], start=True, stop=True)
                    s1 = s128.tile([128, 128], F32, tag="ks")
                    t1 = s128.tile([128, 128], F32, tag="pkt")
                    nc.scalar.copy(s1[:], a12[:, 0:128])  # DVE can't read 2 PSUM operands
                    nc.vector.tensor_mul(t1[:], s1[:], a12[:, 128:256])
                    nc.vector.tensor_mul(phik[:, tau, g, :], t1[:], t1[:])

                # M-cross + s: pg[(h,r), (h',d)|s] = sum_t phik[t,(h,r)] vT[t,(h',d)|1]
                pg = ppg.tile([128, 130], F32, tag="pg")
                for tau in range(NT):
                    nc.tensor.matmul(pg[:, 0:128], lhsT=phik[:, tau, g, :],
                                     rhs=vT_sb[:, tau, g, 0:128],
                                     start=(tau == 0), stop=(tau == NT - 1))
                for tau in range(NT):
                    nc.tensor.matmul(pg[:, 128:130], lhsT=phik[:, tau, g, :],
                                     rhs=vT_sb[:, tau, g, 128:130],
                                     start=(tau == 0), stop=(tau == NT - 1),
                                     skip_group_check=True)
                for hg in range(4):
                    sl = slice(32 * hg, 32 * hg + 32)
                    nc.vector.tensor_copy(mcomp[sl, g, 0:32], pg[sl, sl])
                nc.vector.tensor_copy(mcomp[:, g, 32:33], pg[:, 128:129])

            # ---- AllGather the compact M/s across the 4 L-shards, sum on-chip
            ccin = dpool.tile([128, NG * 33], F32)
            ccout = dpool.tile([4 * 128, NG * 33], F32)
            nc.sync.dma_start(out=ccin[:], in_=mcomp.rearrange("p g f -> p (g f)"))
            nc.gpsimd.collective_compute(
                "AllGather",
                mybir.AluOpType.bypass,
                replica_groups=[[0, 1, 2, 3], [4, 5, 6, 7]],
                ins=[ccin.opt()],
                outs=[ccout.opt()],
            )
            mall = work.tile([128, 4, NG * 33], F32)
            nc.sync.dma_start(out=mall[:],
                              in_=ccout.rearrange("(c p) f -> p c f", p=128))
            mredf = work.tile([128, NG * 33], F32)
            nc.vector.tensor_add(mredf[:], mall[:, 0, :], mall[:, 1, :])
            nc.vector.tensor_add(mredf[:], mredf[:], mall[:, 2, :])
            nc.vector.tensor_add(mredf[:], mredf[:], mall[:, 3, :])
            mred = mredf.rearrange("p (g f) -> p g f", g=NG)

            # ---- q conv (overlaps the collective) ----
            q_sb = work.tile([128, NG, CHUNK], F32R)
            for g in range(NG):
                qps = p512.tile([128, CHUNK], F32, tag="mm")
                idx = 0
                for tap in range(K):
                    for ct in range(NG):
                        nc.tensor.matmul(
                            qps[:],
                            lhsT=qws[:, tap, ct, g * 128:(g + 1) * 128],
                            rhs=xs[:, ct, tap:tap + CHUNK],
                            start=(idx == 0),
                            stop=(idx == K * NG - 1),
                        )
                        idx += 1
                nc.scalar.activation(q_sb[:, g, :], qps[:], AF.Identity,
                                     bias=qbs[:, g:g + 1], scale=1.0)

            # ---- phi_q in [(h,r), t] layout ----
            phiq = work.tile([128, NG, CHUNK], F32R)
            for g in range(NG):
                a1q = p512.tile([128, CHUNK], F32, tag="mm")
                a2q = p512.tile([128, CHUNK], F32, tag="mm")
                nc.tensor.matmul(a1q[:], lhsT=bdgs[:, 0, :], rhs=q_sb[:, g, :], start=True, stop=True)
                nc.tensor.matmul(a2q[:], lhsT=bdgs[:, 1, :], rhs=q_sb[:, g, :], start=True, stop=True)
                sq = s512.tile([128, CHUNK], F32, tag="sq")
                tq = s512.tile([128, CHUNK], F32, tag="tq")
                nc.scalar.copy(sq[:], a1q[:])  # DVE can't read 2 PSUM operands
                nc.vector.tensor_mul(tq[:], sq[:], a2q[:])
                nc.vector.tensor_mul(phiq[:, g, :], tq[:], tq[:])

            # ---- block-diag M (bd) and row-replicated s (srep) from reduced stats
            bd = work.tile([128, NG, 128], F32R)
            srep = work.tile([128, NG, 128], F32R)
            nc.scalar.copy(bd.rearrange("p g f -> p (g f)"), zerof[:])
            nc.scalar.copy(srep.rearrange("p g f -> p (g f)"), zerof[:])
            for g in range(NG):
                for hg in range(4):
                    sl = slice(32 * hg, 32 * hg + 32)
                    nc.vector.tensor_copy(bd[sl, g, sl], mred[sl, g, 0:32])
                    nc.vector.tensor_scalar_mul(srep[sl, g, sl], onesf[sl, 0:32],
                                                mred[sl, g, 32:33])

            # ---- numerator / denominator / o ----
            o_sb = work.tile([128, NG, CHUNK], F32R)
            for g in range(NG):
                nps = p512.tile([128, CHUNK], F32, tag="mm")
                dps = p512.tile([128, CHUNK], F32, tag="mm")
                nc.tensor.matmul(nps[:], lhsT=bd[:, g, :], rhs=phiq[:, g, :], start=True, stop=True)
                nc.tensor.matmul(dps[:], lhsT=srep[:, g, :], rhs=phiq[:, g, :], start=True, stop=True)
                dsb = s512.tile([128, CHUNK], F32, tag="dsb")
                rsb = s512.tile([128, CHUNK], F32, tag="rsb")
                nc.vector.tensor_scalar_add(dsb[:], dps[:], 1e-6)
                nc.vector.reciprocal_approx_fast(rsb[:], dsb[:])
                nc.vector.tensor_mul(o_sb[:, g, :], nps[:], rsb[:])

            # ---- output projection: out[t, i] = sum_c o[c, t] projT[c, i] + pb[i]
            outs_sb = work.tile([128, NT, DIM], F32)
            for tau in range(NT):
                ops = p512.tile([128, DIM], F32, tag="mm")
                for g in range(NG):
                    nc.tensor.matmul(
                        ops[:],
                        lhsT=o_sb[:, g, tau * 128:(tau + 1) * 128],
                        rhs=pws[:, g, :],
                        start=(g == 0),
                        stop=(g == NG - 1),
                    )
                nc.vector.tensor_add(outs_sb[:, tau, :], ops[:], pbs[:])
            nc.sync.dma_start(out=out.rearrange("(tau p) i -> p tau i", p=128),
                              in_=outs_sb[:])

    nc.compile()
    return nc


_NC_CACHE = {}


def _get_nc():
    if "nc" not in _NC_CACHE:
        _NC_CACHE["nc"] = _build_nc()
    return _NC_CACHE["nc"]


def _numpy_fallback(x, q_w, q_b, k_w, k_b, v_w, proj_w, proj_b,
                    gamma_q, beta_q, gamma_k, beta_k, G1q, G2q, G1k, G2k):
    # Exact-reference path, only taken for parameter regimes the Bass kernel
    # doesn't specialize for (beta != 0). Never hit with the shipped setup.
    xp = np.pad(x, ((0, 0), (0, 0), (K - 1, 0)))
    def conv(xx, w):
        o = np.zeros((B, w.shape[0], L), np.float32)
        for t in range(w.shape[2]):
            o += np.einsum("oi,bit->bot", w[:, :, t], xx[:, :, t:t + L])
        return o
    q = conv(xp, q_w) + q_b[None, :, None]
    k = conv(xp, k_w) + k_b[None, :, None]
    v = np.einsum("oi,bit->bot", v_w[:, :, 0], x)
    def shp(t):
        return t.reshape(B, H, HD, L).transpose(0, 1, 3, 2)
    q, k, v = shp(q), shp(k), shp(v)
    q = gamma_q * q + beta_q
    k = gamma_k * k + beta_k
    def sk(t, G1, G2):
        half = (t @ G1) * (t @ G2) / math.sqrt(R)
        return half * half
    pq, pk = sk(q, G1q, G2q), sk(k, G1k, G2k)
    M = np.einsum("bhlr,bhld->bhrd", pk, v)
    s = pk.sum(axis=2)
    num = np.einsum("bhlr,bhrd->bhld", pq, M)
    den = np.einsum("bhlr,bhr->bhl", pq, s) + 1e-6
    o = num / den[..., None]
    o = o.transpose(0, 1, 3, 2).reshape(B, DIM, L).transpose(0, 2, 1)
    return (o @ proj_w.T + proj_b).astype(np.float32)


def kernel(**inputs):
    f = lambda k_: np.ascontiguousarray(np.asarray(inputs[k_], dtype=np.float32))
    x, q_w, q_b, k_w, k_b = f("x"), f("q_w"), f("q_b"), f("k_w"), f("k_b")
    v_w, proj_w, proj_b = f("v_w"), f("proj_w"), f("proj_b")
    G1q, G2q, G1k, G2k = f("G1q"), f("G2q"), f("G1k"), f("G2k")
    gamma_q = float(np.asarray(inputs["gamma_q"]).reshape(-1)[0])
    beta_q = float(np.asarray(inputs["beta_q"]).reshape(-1)[0])
    gamma_k = float(np.asarray(inputs["gamma_k"]).reshape(-1)[0])
    beta_k = float(np.asarray(inputs["beta_k"]).reshape(-1)[0])

    if beta_q != 0.0 or beta_k != 0.0:
        return _numpy_fallback(x, q_w, q_b, k_w, k_b, v_w, proj_w, proj_b,
                               gamma_q, beta_q, gamma_k, beta_k,
                               G1q, G2q, G1k, G2k)

    # host-side weight prep
    cfac = R ** (-0.25)
    g_mats = [G1q * (gamma_q * cfac), G2q * (gamma_q * cfac),
              G1k * (gamma_k * cfac), G2k * (gamma_k * cfac)]
    bdg = np.zeros((4, 128, 128), np.float32)
    for gi, gm in enumerate(g_mats):
        for i in range(4):
            bdg[gi, 32 * i:32 * i + 32, 32 * i:32 * i + 32] = gm
    common = dict(
        qw=np.ascontiguousarray(q_w.transpose(2, 1, 0)),    # [K, cin, cout]
        kw=np.ascontiguousarray(k_w.transpose(2, 1, 0)),
        vw=np.ascontiguousarray(v_w[:, :, 0].T),            # [cin, cout]
        pw=np.ascontiguousarray(proj_w.T),                  # [c, i]
        bdg=bdg,
        qb=np.ascontiguousarray(q_b.reshape(NG, 128)),
        kb=np.ascontiguousarray(k_b.reshape(NG, 128)),
        pb=proj_b,
    )
    xpad = np.pad(x, ((0, 0), (0, 0), (K - 1, 0)))          # [B, DIM, L+2]
    in_maps = []
    for core in range(NCORES):
        b, j = divmod(core, 4)
        xsl = np.ascontiguousarray(xpad[b][:, j * CHUNK: j * CHUNK + CHUNK + K - 1])
        in_maps.append(dict(x_sl=xsl, **common))

    nc = _get_nc()
    res = run_bass_kernel_spmd(nc, in_maps, list(range(NCORES)),
                               trace=bool(os.environ.get("BASS_TRACE")))
    kernel.last_results = res

    out = np.empty((B, L, DIM), np.float32)
    for core in range(NCORES):
        b, j = divmod(core, 4)
        out[b, j * CHUNK:(j + 1) * CHUNK, :] = res.results[core]["out"]
    return out


# revision 21
# speedup vs baseline: 1.9761x; 1.0648x over previous
"""Trainium2 Bass kernel for Conv1D-MHSA with p4-sketch linear attention.

Math: the reference computes
    scores = phi_q @ phi_k^T            # [B,H,L,L]
    attn   = scores / (scores.sum(-1) + 1e-6)
    o      = attn @ v
Since phi_* >= 0 and the normalizer is a plain row sum, this reassociates
exactly to linear attention:
    M   = phi_k^T @ v                   # [R, HD] per (b, h)
    s   = phi_k.sum(axis=L)             # [R]
    o   = (phi_q @ M) / (phi_q @ s + 1e-6)
which removes the [L, L] score materialization entirely.

Sharding: 8 cores = (batch b in {0,1}) x (L-quarter j in {0..3}).
Each core runs the causal convs (2-col halo), sketches, and the output
projection for its 512 positions.  The only cross-core data is the
[R x (HD+1)] per-head M/s reduction over L -> one 68 KB AllGather per
batch group ([[0,1,2,3],[4,5,6,7]]) + on-chip shard sum.

Layout trick: heads are processed 4-at-a-time (4 x HD=32 = 128 partitions)
using block-diagonal sketch matrices (gamma and R^-1/4 folded in on host),
so every matmul is a full 128-contract matmul.  Matmuls run in float32r
(single-pass fp32) instead of float32 (two-pass).
"""

import math
import os

import numpy as np

import concourse.bass as bass
import concourse.tile as tile
from concourse import bacc, mybir
from concourse.bass_utils import run_bass_kernel_spmd

F32 = mybir.dt.float32
F32R = mybir.dt.float32r

B, DIM, L = 2, 512, 2048
H, HD, R, K = 16, 32, 32, 3
NCORES = 8
CHUNK = L // 4          # 512 positions per core
NG = DIM // 128         # 4 channel groups (4 heads each)
NT = CHUNK // 128       # 4 position tiles per core
AF = mybir.ActivationFunctionType


def _build_nc():
    nc = bacc.Bacc("TRN2", debug=False, num_devices=NCORES)

    # Per-core DRAM I/O (names must match in_maps keys)
    x_sl = nc.dram_tensor("x_sl", [DIM, CHUNK + K - 1], F32R, kind="ExternalInput").ap()
    qw = nc.dram_tensor("qw", [K, DIM, DIM], F32R, kind="ExternalInput").ap()
    kw = nc.dram_tensor("kw", [K, DIM, DIM], F32R, kind="ExternalInput").ap()
    vw = nc.dram_tensor("vw", [DIM, DIM], F32R, kind="ExternalInput").ap()
    pw = nc.dram_tensor("pw", [DIM, DIM], F32R, kind="ExternalInput").ap()
    bdg = nc.dram_tensor("bdg", [4, 128, 128], F32R, kind="ExternalInput").ap()
    qb = nc.dram_tensor("qb", [NG, 128], F32, kind="ExternalInput").ap()
    kb = nc.dram_tensor("kb", [NG, 128], F32, kind="ExternalInput").ap()
    pb = nc.dram_tensor("pb", [DIM], F32, kind="ExternalInput").ap()
    out = nc.dram_tensor("out", [CHUNK, DIM], F32, kind="ExternalOutput").ap()

    with tile.TileContext(nc) as tc:
        with (
            tc.tile_pool(name="consts", bufs=1) as consts,
            tc.tile_pool(name="work", bufs=1) as work,
            tc.tile_pool(name="s128", bufs=2) as s128,
            tc.tile_pool(name="s512", bufs=2) as s512,
            tc.tile_pool(name="p512", bufs=4, space="PSUM") as p512,
            tc.tile_pool(name="p128", bufs=2, space="PSUM") as p128,
            tc.tile_pool(name="ppg", bufs=2, space="PSUM") as ppg,
            tc.tile_pool(name="dram", bufs=1, space="DRAM") as dpool,
        ):
            # ---- loads, in need-order (sync ring drains FIFO); k-path first
            xs = consts.tile([128, NG, CHUNK + K - 1], F32R)
            nc.sync.dma_start(out=xs[:], in_=x_sl.rearrange("(ct p) t -> p ct t", p=128))
            kws = consts.tile([128, K, NG, DIM], F32R)
            nc.sync.dma_start(out=kws[:], in_=kw.rearrange("k (ct p) n -> p k ct n", p=128))
            bdgs = consts.tile([128, 4, 128], F32R)
            nc.sync.dma_start(out=bdgs[:], in_=bdg.rearrange("g p n -> p g n"))
            kbs = consts.tile([128, NG], F32)
            nc.sync.dma_start(out=kbs[:], in_=kb.rearrange("ct p -> p ct"))
            vws = consts.tile([128, NG, DIM], F32R)
            nc.sync.dma_start(out=vws[:], in_=vw.rearrange("(ct p) n -> p ct n", p=128))
            qws = consts.tile([128, K, NG, DIM], F32R)
            nc.sync.dma_start(out=qws[:], in_=qw.rearrange("k (ct p) n -> p k ct n", p=128))
            qbs = consts.tile([128, NG], F32)
            nc.sync.dma_start(out=qbs[:], in_=qb.rearrange("ct p -> p ct"))

            # f32 scratch for filling f32r tiles (memset can't write f32r)
            onesf = consts.tile([128, 32], F32)
            nc.vector.memset(onesf[:], 1.0)
            zerof = consts.tile([128, 512], F32)
            nc.vector.memset(zerof[:], 0.0)

            # ---- k conv per group, then phi_k + M/s partial for that group ----
            k_sb = work.tile([128, NG, CHUNK], F32R)
            phik = work.tile([128, NT, NG, 128], F32R)
            vT_sb = work.tile([128, NT, NG, 130], F32R)
            nc.scalar.copy(vT_sb[:, :, :, 128:130],
                           onesf[:, 0:32].rearrange("p (a b c) -> p a b c", a=NT, b=NG))
            mcomp = work.tile([128, NG, 33], F32)

            def conv_group(g, wsb, bsb, dst):
                ps = p512.tile([128, CHUNK], F32, tag="mm")
                idx = 0
                for tap in range(K):
                    for ct in range(NG):
                        nc.tensor.matmul(
                            ps[:],
                            lhsT=wsb[:, tap, ct, g * 128:(g + 1) * 128],
                            rhs=xs[:, ct, tap:tap + CHUNK],
                            start=(idx == 0),
                            stop=(idx == K * NG - 1),
                        )
                        idx += 1
                nc.scalar.activation(dst[:, g, :], ps[:], AF.Identity,
                                     bias=bsb[:, g:g + 1], scale=1.0)

            # v^T (needed by the M matmuls): vT[t, c] = sum_cin x[cin, t] WvT[cin, c]
            def vt_tile(tau):
                vps = p512.tile([128, DIM], F32, tag="mm")
                for ct in range(NG):
                    nc.tensor.matmul(
                        vps[:],
                        lhsT=xs[:, ct, (K - 1) + tau * 128:(K - 1) + tau * 128 + 128],
                        rhs=vws[:, ct, :],
                        start=(ct == 0),
                        stop=(ct == NG - 1),
                    )
                for g in range(NG):
                    nc.vector.tensor_copy(vT_sb[:, tau, g, 0:128],
                                          vps[:, g * 128:(g + 1) * 128])

            def kpath_group(g):
                conv_group(g, kws, kbs, k_sb)
                # phi_k in [t, (h,r)] layout: ((k^T BDG1k) * (k^T BDG2k))^2
                for tau in range(NT):
                    a12 = p128.tile([128, 256], F32, tag="ak")
                    ksl = k_sb[:, g, tau * 128:(tau + 1) * 128]
                    nc.tensor.matmul(a12[:], lhsT=ksl, rhs=bdgs[:, 2:4, :], start=True, stop=True)
                    s1 = s128.tile([128, 128], F32, tag="ks")
                    t1 = s128.tile([128, 128], F32, tag="pkt")
                    nc.scalar.copy(s1[:], a12[:, 0:128])  # DVE can't read 2 PSUM operands
                    nc.vector.tensor_mul(t1[:], s1[:], a12[:, 128:256])
                    nc.vector.tensor_mul(phik[:, tau, g, :], t1[:], t1[:])
                # M-cross + s: pg[(h,r), (h',d)|s] = sum_t phik[t,(h,r)] vT[t,(h',d)|1]
                pg = ppg.tile([128, 130], F32, tag="pg")
                for tau in range(NT):
                    nc.tensor.matmul(pg[:, 0:128], lhsT=phik[:, tau, g, :],
                                     rhs=vT_sb[:, tau, g, 0:128],
                                     start=(tau == 0), stop=(tau == NT - 1))
                for tau in range(NT):
                    nc.tensor.matmul(pg[:, 128:130], lhsT=phik[:, tau, g, :],
                                     rhs=vT_sb[:, tau, g, 128:130],
                                     start=(tau == 0), stop=(tau == NT - 1),
                                     skip_group_check=True)
                for hg in range(4):
                    sl = slice(32 * hg, 32 * hg + 32)
                    nc.vector.tensor_copy(mcomp[sl, g, 0:32], pg[sl, sl])
                nc.vector.tensor_copy(mcomp[:, g, 32:33], pg[:, 128:129])

            # two half-collectives so the first fires as soon as groups 0-1 finish
            HALF = 2 * 33
            ccin = [dpool.tile([128, HALF], F32, name=f"ccin{h}") for h in range(2)]
            ccout = [dpool.tile([4 * 128, HALF], F32, name=f"ccout{h}") for h in range(2)]
            mall = [work.tile([128, 4, HALF], F32, name=f"mall{h}") for h in range(2)]
            mredf = work.tile([128, NG * 33], F32)

            for tau in range(NT):
                vt_tile(tau)
            kpath_group(0)
            kpath_group(1)

            def half_collective(h):
                nc.sync.dma_start(
                    out=ccin[h][:],
                    in_=mcomp[:, 2 * h:2 * h + 2, :].rearrange("p g f -> p (g f)"))
                nc.gpsimd.collective_compute(
                    "AllGather",
                    mybir.AluOpType.bypass,
                    replica_groups=[[0, 1, 2, 3], [4, 5, 6, 7]],
                    ins=[ccin[h].opt()],
                    outs=[ccout[h].opt()],
                )
                nc.sync.dma_start(out=mall[h][:],
                                  in_=ccout[h].rearrange("(c p) f -> p c f", p=128))
                dstf = mredf[:, 2 * h * 33:(2 * h + 2) * 33]
                nc.vector.tensor_add(dstf, mall[h][:, 0, :], mall[h][:, 1, :])
                nc.vector.tensor_add(dstf, dstf, mall[h][:, 2, :])
                nc.vector.tensor_add(dstf, dstf, mall[h][:, 3, :])

            half_collective(0)
            kpath_group(2)
            kpath_group(3)
            half_collective(1)
            mred = mredf.rearrange("p (g f) -> p g f", g=NG)

            # ---- q path (overlaps the collectives) ----
            q_sb = work.tile([128, NG, CHUNK], F32R)
            phiq = work.tile([128, NG, CHUNK], F32R)
            for g in range(NG):
                conv_group(g, qws, qbs, q_sb)
                a1q = p512.tile([128, CHUNK], F32, tag="mm")
                a2q = p512.tile([128, CHUNK], F32, tag="mm")
                nc.tensor.matmul(a1q[:], lhsT=bdgs[:, 0, :], rhs=q_sb[:, g, :], start=True, stop=True)
                nc.tensor.matmul(a2q[:], lhsT=bdgs[:, 1, :], rhs=q_sb[:, g, :], start=True, stop=True)
                sq = s512.tile([128, CHUNK], F32, tag="sq")
                tq = s512.tile([128, CHUNK], F32, tag="tq")
                nc.scalar.copy(sq[:], a1q[:])  # DVE can't read 2 PSUM operands
                nc.vector.tensor_mul(tq[:], sq[:], a2q[:])
                nc.vector.tensor_mul(phiq[:, g, :], tq[:], tq[:])

            # load proj weights late (ring FIFO: don't delay k/q weights)
            pws = consts.tile([128, NG, DIM], F32R)
            nc.sync.dma_start(out=pws[:], in_=pw.rearrange("(g p) n -> p g n", p=128))
            pbs = consts.tile([128, DIM], F32)
            pb_bcast = bass.AP(tensor=pb.tensor, offset=pb.offset, ap=[[0, 128], *pb.ap])
            nc.sync.dma_start(out=pbs[:], in_=pb_bcast)

            # ---- per group: bd/srep from reduced stats, then num/den/o ----
            bd = work.tile([128, NG, 128], F32R)
            srep = work.tile([128, NG, 128], F32R)
            nc.scalar.copy(bd.rearrange("p g f -> p (g f)"), zerof[:])
            nc.scalar.copy(srep.rearrange("p g f -> p (g f)"), zerof[:])
            o_sb = work.tile([128, NG, CHUNK], F32R)
            for g in range(NG):
                for hg in range(4):
                    sl = slice(32 * hg, 32 * hg + 32)
                    nc.vector.tensor_copy(bd[sl, g, sl], mred[sl, g, 0:32])
                    nc.vector.tensor_scalar_mul(srep[sl, g, sl], onesf[sl, 0:32],
                                                mred[sl, g, 32:33])
                nps = p512.tile([128, CHUNK], F32, tag="mm")
                dps = p512.tile([128, CHUNK], F32, tag="mm")
                nc.tensor.matmul(nps[:], lhsT=bd[:, g, :], rhs=phiq[:, g, :], start=True, stop=True)
                nc.tensor.matmul(dps[:], lhsT=srep[:, g, :], rhs=phiq[:, g, :], start=True, stop=True)
                dsb = s512.tile([128, CHUNK], F32, tag="dsb")
                rsb = s512.tile([128, CHUNK], F32, tag="rsb")
                nc.vector.tensor_scalar_add(dsb[:], dps[:], 1e-6)
                nc.vector.reciprocal_approx_fast(rsb[:], dsb[:])
                nc.vector.tensor_mul(o_sb[:, g, :], nps[:], rsb[:])

            # ---- output projection: out[t, i] = sum_c o[c, t] projT[c, i] + pb[i]
            outs_sb = work.tile([128, NT, DIM], F32)
            for tau in range(NT):
                ops = p512.tile([128, DIM], F32, tag="mm")
                for g in range(NG):
                    nc.tensor.matmul(
                        ops[:],
                        lhsT=o_sb[:, g, tau * 128:(tau + 1) * 128],
                        rhs=pws[:, g, :],
                        start=(g == 0),
                        stop=(g == NG - 1),
                    )
                nc.vector.tensor_add(outs_sb[:, tau, :], ops[:], pbs[:])
                nc.sync.dma_start(out=out[tau * 128:(tau + 1) * 128, :],
                                  in_=outs_sb[:, tau, :])

    nc.compile()
    return nc


_NC_CACHE = {}


def _get_nc():
    if "nc" not in _NC_CACHE:
        _NC_CACHE["nc"] = _build_nc()
    return _NC_CACHE["nc"]


def _numpy_fallback(x, q_w, q_b, k_w, k_b, v_w, proj_w, proj_b,
                    gamma_q, beta_q, gamma_k, beta_k, G1q, G2q, G1k, G2k):
    # Exact-reference path, only taken for parameter regimes the Bass kernel
    # doesn't specialize for (beta != 0). Never hit with the shipped setup.
    xp = np.pad(x, ((0, 0), (0, 0), (K - 1, 0)))
    def conv(xx, w):
        o = np.zeros((B, w.shape[0], L), np.float32)
        for t in range(w.shape[2]):
            o += np.einsum("oi,bit->bot", w[:, :, t], xx[:, :, t:t + L])
        return o
    q = conv(xp, q_w) + q_b[None, :, None]
    k = conv(xp, k_w) + k_b[None, :, None]
    v = np.einsum("oi,bit->bot", v_w[:, :, 0], x)
    def shp(t):
        return t.reshape(B, H, HD, L).transpose(0, 1, 3, 2)
    q, k, v = shp(q), shp(k), shp(v)
    q = gamma_q * q + beta_q
    k = gamma_k * k + beta_k
    def sk(t, G1, G2):
        half = (t @ G1) * (t @ G2) / math.sqrt(R)
        return half * half
    pq, pk = sk(q, G1q, G2q), sk(k, G1k, G2k)
    M = np.einsum("bhlr,bhld->bhrd", pk, v)
    s = pk.sum(axis=2)
    num = np.einsum("bhlr,bhrd->bhld", pq, M)
    den = np.einsum("bhlr,bhr->bhl", pq, s) + 1e-6
    o = num / den[..., None]
    o = o.transpose(0, 1, 3, 2).reshape(B, DIM, L).transpose(0, 2, 1)
    return (o @ proj_w.T + proj_b).astype(np.float32)


def kernel(**inputs):
    f = lambda k_: np.ascontiguousarray(np.asarray(inputs[k_], dtype=np.float32))
    x, q_w, q_b, k_w, k_b = f("x"), f("q_w"), f("q_b"), f("k_w"), f("k_b")
    v_w, proj_w, proj_b = f("v_w"), f("proj_w"), f("proj_b")
    G1q, G2q, G1k, G2k = f("G1q"), f("G2q"), f("G1k"), f("G2k")
    gamma_q = float(np.asarray(inputs["gamma_q"]).reshape(-1)[0])
    beta_q = float(np.asarray(inputs["beta_q"]).reshape(-1)[0])
    gamma_k = float(np.asarray(inputs["gamma_k"]).reshape(-1)[0])
    beta_k = float(np.asarray(inputs["beta_k"]).reshape(-1)[0])

    if beta_q != 0.0 or beta_k != 0.0:
        return _numpy_fallback(x, q_w, q_b, k_w, k_b, v_w, proj_w, proj_b,
                               gamma_q, beta_q, gamma_k, beta_k,
                               G1q, G2q, G1k, G2k)

    # host-side weight prep
    cfac = R ** (-0.25)
    g_mats = [G1q * (gamma_q * cfac), G2q * (gamma_q * cfac),
              G1k * (gamma_k * cfac), G2k * (gamma_k * cfac)]
    bdg = np.zeros((4, 128, 128), np.float32)
    for gi, gm in enumerate(g_mats):
        for i in range(4):
            bdg[gi, 32 * i:32 * i + 32, 32 * i:32 * i + 32] = gm
    common = dict(
        qw=np.ascontiguousarray(q_w.transpose(2, 1, 0)),    # [K, cin, cout]
        kw=np.ascontiguousarray(k_w.transpose(2, 1, 0)),
        vw=np.ascontiguousarray(v_w[:, :, 0].T),            # [cin, cout]
        pw=np.ascontiguousarray(proj_w.T),                  # [c, i]
        bdg=bdg,
        qb=np.ascontiguousarray(q_b.reshape(NG, 128)),
        kb=np.ascontiguousarray(k_b.reshape(NG, 128)),
        pb=proj_b,
    )
    xpad = np.pad(x, ((0, 0), (0, 0), (K - 1, 0)))          # [B, DIM, L+2]
    in_maps = []
    for core in range(NCORES):
        b, j = divmod(core, 4)
        xsl = np.ascontiguousarray(xpad[b][:, j * CHUNK: j * CHUNK + CHUNK + K - 1])
        in_maps.append(dict(x_sl=xsl, **common))

    nc = _get_nc()
    res = run_bass_kernel_spmd(nc, in_maps, list(range(NCORES)),
                               trace=bool(os.environ.get("BASS_TRACE")))
    kernel.last_results = res

    out = np.empty((B, L, DIM), np.float32)
    for core in range(NCORES):
        b, j = divmod(core, 4)
        out[b, j * CHUNK:(j + 1) * CHUNK, :] = res.results[core]["out"]
    return out
